# revision 1
# baseline (speedup 1.0000x reference)
"""IterNorm (iterative whitening normalization) Trainium2 kernel, 8-core SPMD.

Algorithm (matches reference exactly, single pass over data for stats):
  x = X.transpose(1,0,2,3).reshape(C, m)          # C=256, m = N*H*W
  S = x @ x.T, rs = x @ 1                          (per-core partials, AllReduce)
  mu = rs/m; std = sqrt((diag(S)-m mu^2)/(m-1)) + 1e-5
  sigma = EPS I + (S - m mu mu^T)/(m std_i std_j)
  sigma_N = sigma / trace(sigma);  Newton-Schulz x5 -> P
  wm = P sqrt(1/trace);  out = A @ x + (-A@mu),  A = wm diag(1/std)

Sharding: data-parallel over batch N (8 images per core), AllReduce of
(S, rowsum) [128x520 f32], replicated stats + Newton-Schulz on every core.
"""

import numpy as np

import concourse.bass as bass
import concourse.bacc as bacc
import concourse.tile as tile
import concourse.mybir as mybir
from concourse.bass import ds, ts
from concourse.bass_isa import ReduceOp
from concourse.bass_utils import run_bass_kernel_spmd
from concourse.masks import make_identity

F32 = mybir.dt.float32
F32R = mybir.dt.float32r
AX = mybir.AxisListType
ALU = mybir.AluOpType
ACT = mybir.ActivationFunctionType

N_CORES = 8
N, C, H, W = 64, 256, 56, 56
HW = H * W              # 3136
NPC = N // N_CORES      # 8 images per core
M_LOC = NPC * HW        # 25088
M_TOT = N * HW          # 200704
EPS = 0.001
EPS_BN = 1e-5
T_NS = 5

RES_IMGS = 6            # images kept resident in SBUF between the two passes
P1C = 112               # pass-1 m-chunk (28 per image)
P2C = 392               # pass-2 m-chunk (8 per image)
STREAM_W = 784          # streamed-image tile width (7 p1-chunks / 2 p2-chunks)
USE_F32R_BIG = True     # fp32r (tf32-ish) for the two big matmuls
USE_F32R_NS = False      # fp32r for the Newton-Schulz matmuls


def _r(ap):
    return ap.bitcast(F32R)


def _build(reps: int = 1):
    """Build + compile the SPMD program. reps>1 wraps pass1 / stats+NS / pass2
    each in a For_i loop for wall-clock delta timing (numerics of S accumulate
    across reps; only used for perf measurement)."""
    nc = bacc.Bacc(
        "TRN2",
        target_bir_lowering=False,
        debug=False,
        enable_asserts=False,
        num_devices=N_CORES,
    )
    x = nc.dram_tensor("x", [NPC * C, HW], F32R, kind="ExternalInput").ap()
    y = nc.dram_tensor("y", [NPC * C, HW], F32, kind="ExternalOutput").ap()

    with tile.TileContext(nc) as tc:
        _emit(nc, tc, x, y, reps)
    nc.compile()
    return nc


def _emit(nc, tc, x, y, reps):
    import contextlib

    ctx = contextlib.ExitStack()
    with ctx:
        consts = ctx.enter_context(tc.tile_pool(name="consts", bufs=1))
        resid = ctx.enter_context(tc.tile_pool(name="resid", bufs=1))
        stats = ctx.enter_context(tc.tile_pool(name="stats", bufs=1))
        smalls = ctx.enter_context(tc.tile_pool(name="smalls", bufs=2))
        dram = ctx.enter_context(tc.tile_pool(name="dram", bufs=1, space="DRAM"))

        # ---- constants ----
        ones = consts.tile([128, 1], F32)
        nc.vector.memset(ones, 1.0)
        # eps_eye: [128, 512]; block b holds EPS * delta(j, 128*b + i)
        eps_eye = consts.tile([128, 512], F32)
        nc.gpsimd.memset(eps_eye, 0.0)
        nc.gpsimd.affine_select(
            out=eps_eye[:, 0:256], in_=eps_eye[:, 0:256],
            compare_op=ALU.not_equal, fill=EPS,
            base=0, pattern=[[-1, 256]], channel_multiplier=1,
        )
        nc.gpsimd.affine_select(
            out=eps_eye[:, 256:512], in_=eps_eye[:, 256:512],
            compare_op=ALU.not_equal, fill=EPS,
            base=128, pattern=[[-1, 256]], channel_multiplier=1,
        )

        # ---- resident image tiles ----
        xres = []  # [img][block] -> tile [128, HW]
        for n in range(RES_IMGS):
            xres.append([
                resid.tile([128, HW], F32R, tag=f"xr{n}_{b}", name=f"xr{n}_{b}")
                for b in range(2)
            ])

        s_sb = stats.tile([128, 520], F32)
        g_sb = stats.tile([128, 520], F32)
        ar_in = dram.tile([128, 520], F32)
        ar_out = dram.tile([128, 520], F32)
        drows = dram.tile([2, 256], F32)

        sig = stats.tile([128, 512], F32)     # sigma, then sigma_N (in place)
        Pm = stats.tile([128, 512], F32)      # Newton-Schulz iterate
        M1 = stats.tile([128, 512], F32)
        M2 = stats.tile([128, 512], F32)
        A_T = stats.tile([128, 512], F32R)
        tmp512 = stats.tile([128, 512], F32)
        rowv = stats.tile([1, 512], F32)      # [q_row | rstd_row] each 256
        vec2 = stats.tile([128, 2 * 8], F32)  # packed small vectors
        # column layout in vec2:
        mu_v = vec2[:, 0:2]
        d_v = vec2[:, 2:4]
        std_v = vec2[:, 4:6]
        rstd_v = vec2[:, 6:8]
        q_v = vec2[:, 8:10]
        rstdm_v = vec2[:, 10:12]
        acol_v = vec2[:, 12:14]
        negb_v = vec2[:, 14:16]
        tr_v = vec2[:, 0:1]  # reuse later (mu no longer needed then? keep separate)
        tsum_v = smalls.tile([128, 1], F32, tag="tsum")
        tr_t = smalls.tile([128, 1], F32, tag="tr")
        ti_t = smalls.tile([128, 1], F32, tag="ti")
        tis_t = smalls.tile([128, 1], F32, tag="tis")
        musq_t = smalls.tile([128, 2], F32, tag="musq")
        tmp256 = stats.tile([128, 256], F32)
        rstd_bc = stats.tile([128, 256], F32)
        dummy = stats.tile([128, 1], F32)
        ones_row = consts.tile([1, 128], F32)
        nc.vector.memset(ones_row, 1.0)
        ident32 = consts.tile([128, 128], F32)
        make_identity(nc, ident32)

        # =========================================================
        # PASS 1: load x, accumulate S = x x^T and rowsums on PE
        # =========================================================
        def pass1_body(stream, xtp, ps_tp, s_ps, rs_ps, _iv=None):
            first = [True]

            def do_chunks(xb0, xb1, s_lo, s_hi, last_img):
                for s in range(s_lo, s_hi, P1C):
                    tpA = ps_tp.tile([128, 128], F32, tag="tpA")
                    tpB = ps_tp.tile([128, 128], F32, tag="tpB")
                    nc.tensor.transpose(tpA[:P1C, :], xb0[:, ds(s, P1C)].bitcast(F32), ident32)
                    nc.tensor.transpose(tpB[:P1C, :], xb1[:, ds(s, P1C)].bitcast(F32), ident32)
                    xt = xtp.tile([128, 256], F32R, tag="xt")
                    nc.vector.tensor_copy(xt[:P1C, 0:128], tpA[:P1C, :])
                    nc.scalar.copy(xt[:P1C, 128:256], tpB[:P1C, :])
                    st = first[0]
                    sp = last_img and (s + P1C >= s_hi)
                    xta, xtf = xt[:P1C, :], xt[:, :]
                    nc.tensor.matmul(
                        s_ps[0], xta[:, 0:128], xtf[:P1C, 0:256],
                        start=st, stop=sp, skip_group_check=True)
                    nc.tensor.matmul(
                        s_ps[1], xta[:, 128:256], xtf[:P1C, 0:256],
                        start=st, stop=sp, skip_group_check=True)
                    xtc = xt[:P1C, :].bitcast(F32)
                    nc.tensor.matmul(
                        rs_ps[0], xtc[:, 0:128], ones[:P1C, :],
                        start=st, stop=sp, skip_group_check=True)
                    nc.tensor.matmul(
                        rs_ps[1], xtc[:, 128:256], ones[:P1C, :],
                        start=st, stop=sp, skip_group_check=True)
                    first[0] = False

            for n in range(NPC):
                last = n == NPC - 1
                if n < RES_IMGS:
                    for b in range(2):
                        nc.sync.dma_start(
                            out=xres[n][b], in_=x[ds(n * C + 128 * b, 128), :])
                    do_chunks(xres[n][0], xres[n][1], 0, HW, last)
                else:
                    for w0 in range(0, HW, STREAM_W):
                        xs0 = stream.tile([128, STREAM_W], F32R, tag="xs0")
                        xs1 = stream.tile([128, STREAM_W], F32R, tag="xs1")
                        nc.sync.dma_start(
                            out=xs0, in_=x[ds(n * C, 128), ds(w0, STREAM_W)])
                        nc.sync.dma_start(
                            out=xs1, in_=x[ds(n * C + 128, 128), ds(w0, STREAM_W)])
                        do_chunks(xs0, xs1, 0, STREAM_W,
                                  last and (w0 + STREAM_W >= HW))

        with (
            tc.tile_pool(name="stream1", bufs=3) as stream1,
            tc.tile_pool(name="xtp", bufs=4) as xtp,
            tc.tile_pool(name="ps_acc", bufs=1, space="PSUM") as ps_acc,
            tc.tile_pool(name="ps_tp1", bufs=2, space="PSUM") as ps_tp1,
        ):
            s_ps = [ps_acc.tile([128, 256], F32, tag=f"s{b}", name=f"s_ps{b}")
                    for b in range(2)]
            rs_ps = [ps_acc.tile([128, 1], F32, tag=f"rs{b}", name=f"rs_ps{b}")
                     for b in range(2)]
            if reps > 1:
                with tc.For_i(0, reps, 1):
                    pass1_body(stream1, xtp, ps_tp1, s_ps, rs_ps)
            else:
                pass1_body(stream1, xtp, ps_tp1, s_ps, rs_ps)

            # collect S + rowsums into SBUF, AllReduce
            nc.vector.tensor_copy(s_sb[:, 0:256], s_ps[0])
            nc.scalar.copy(s_sb[:, 256:512], s_ps[1])
            nc.vector.tensor_copy(s_sb[:, 512:513], rs_ps[0])
            nc.vector.tensor_copy(s_sb[:, 513:514], rs_ps[1])
            nc.vector.memset(s_sb[:, 514:520], 0.0)
        nc.sync.dma_start(out=ar_in, in_=s_sb)
        nc.gpsimd.collective_compute(
            "AllReduce",
            ALU.add,
            replica_groups=[list(range(N_CORES))],
            ins=[ar_in.opt()],
            outs=[ar_out.opt()],
        )
        nc.sync.dma_start(out=g_sb, in_=ar_out)

        # =========================================================
        # STATS + Newton-Schulz (replicated on every core)
        # =========================================================
        def stats_body(ps_tp, _iv=None):
            G0, G1 = g_sb[:, 0:256], g_sb[:, 256:512]
            # mu = rs / m
            nc.vector.tensor_scalar(
                out=mu_v, in0=g_sb[:, 512:514], scalar1=1.0 / M_TOT, scalar2=None,
                op0=ALU.mult)
            # d = EPS * diag(S)
            for b, G in ((0, G0), (1, G1)):
                nc.vector.tensor_tensor_reduce(
                    out=dummy.broadcast_to([128, 256]),
                    in0=G, in1=eps_eye[:, ds(256 * b, 256)],
                    scale=1.0, scalar=0.0,
                    op0=ALU.mult, op1=ALU.add,
                    accum_out=d_v[:, b:b + 1])
            # v = d/EPS - m*mu^2 ; std = sqrt(v/(m-1)) + EPS_BN
            nc.vector.tensor_mul(musq_t, mu_v, mu_v)
            nc.vector.tensor_scalar(
                out=musq_t, in0=musq_t, scalar1=float(M_TOT), scalar2=None,
                op0=ALU.mult)
            nc.vector.tensor_scalar(
                out=std_v, in0=d_v, scalar1=1.0 / EPS, scalar2=None, op0=ALU.mult)
            nc.vector.tensor_sub(std_v, std_v, musq_t)
            nc.vector.tensor_scalar(
                out=std_v, in0=std_v, scalar1=1.0 / (M_TOT - 1), scalar2=None,
                op0=ALU.mult)
            nc.scalar.sqrt(std_v, std_v)
            nc.vector.tensor_scalar(
                out=std_v, in0=std_v, scalar1=EPS_BN, scalar2=None, op0=ALU.add)
            nc.vector.reciprocal(rstd_v, std_v)
            nc.vector.tensor_mul(q_v, mu_v, rstd_v)
            nc.vector.tensor_scalar(
                out=rstdm_v, in0=rstd_v, scalar1=1.0 / M_TOT, scalar2=None,
                op0=ALU.mult)
            # Row-broadcast matrices via a DRAM bounce: write q/rstd into DRAM
            # in j-order (j = 128*b + i), then read back partition-broadcast.
            drt = drows[:, :]
            nc.gpsimd.dma_start(
                out=bass.AP(tensor=drt.tensor, offset=drt.offset,
                            ap=[[1, 128], [128, 2]]),
                in_=q_v)
            nc.gpsimd.dma_start(
                out=bass.AP(tensor=drt.tensor, offset=drt.offset + 256,
                            ap=[[1, 128], [128, 2]]),
                in_=rstd_v)
            nc.gpsimd.dma_start(
                out=tmp256,
                in_=bass.AP(tensor=drt.tensor, offset=drt.offset,
                            ap=[[0, 128], [1, 256]]))
            nc.gpsimd.dma_start(
                out=rstd_bc,
                in_=bass.AP(tensor=drt.tensor, offset=drt.offset + 256,
                            ap=[[0, 128], [1, 256]]))
            # sigma
            for b, G in ((0, G0), (1, G1)):
                blk = ds(256 * b, 256)
                nc.vector.tensor_scalar_mul(sig[:, blk], G, rstdm_v[:, b:b + 1])
                nc.vector.tensor_mul(sig[:, blk], sig[:, blk], rstd_bc)
                # subtract q_i * q_j: tmp holds q_j broadcast rows
                nc.vector.tensor_scalar(
                    out=tmp512[:, 0:256], in0=tmp256, scalar1=q_v[:, b:b + 1],
                    scalar2=None, op0=ALU.mult)
                nc.vector.tensor_sub(sig[:, blk], sig[:, blk], tmp512[:, 0:256])
                nc.vector.tensor_add(sig[:, blk], sig[:, blk], eps_eye[:, blk])
            # trace
            for b in range(2):
                nc.vector.tensor_tensor_reduce(
                    out=dummy.broadcast_to([128, 256]),
                    in0=sig[:, ds(256 * b, 256)], in1=eps_eye[:, ds(256 * b, 256)],
                    scale=1.0, scalar=0.0, op0=ALU.mult, op1=ALU.add,
                    accum_out=d_v[:, b:b + 1])
            nc.vector.tensor_add(tsum_v, d_v[:, 0:1], d_v[:, 1:2])
            nc.vector.tensor_scalar(
                out=tsum_v, in0=tsum_v, scalar1=1.0 / EPS, scalar2=None,
                op0=ALU.mult)
            nc.gpsimd.partition_all_reduce(tr_t, tsum_v, 128, ReduceOp.add)
            nc.vector.reciprocal(ti_t, tr_t)
            nc.scalar.sqrt(tis_t, ti_t)
            # sigma_N = sigma * trace_inv (in place)
            nc.vector.tensor_scalar_mul(sig, sig, ti_t)

            import os as _os
            if _os.environ.get("STATS_CUT"):
                nc.vector.tensor_scalar(
                    out=A_T, in0=eps_eye, scalar1=1.0 / EPS, scalar2=None,
                    op0=ALU.mult)
                nc.vector.memset(negb_v, 0.0)
                return
            # P = 1.5 I - 0.5 sigma_N
            nc.vector.tensor_scalar(
                out=Pm, in0=sig, scalar1=-0.5, scalar2=None, op0=ALU.mult)
            nc.vector.tensor_scalar(
                out=tmp512, in0=eps_eye, scalar1=1.5 / EPS, scalar2=None,
                op0=ALU.mult)
            nc.vector.tensor_add(Pm, Pm, tmp512)

            def mm256(dst_sb, lhs_sb, rhs_sb):
                """dst = lhs @ rhs for 256x256 symmetric-stored operands."""
                pps = []
                for mb in range(2):
                    pp = ps_tp.tile([128, 256], F32, tag=f"ns{mb}")
                    for kb in range(2):
                        lhsT = lhs_sb[:, ds(256 * kb + 128 * mb, 128)]
                        rhs = rhs_sb[:, ds(256 * kb, 256)]
                        nc.tensor.matmul(
                            pp, lhsT, rhs, start=(kb == 0), stop=(kb == 1),
                            skip_group_check=True)
                    pps.append(pp)
                if dst_sb is not None:
                    nc.vector.tensor_copy(dst_sb[:, 0:256], pps[0])
                    nc.scalar.copy(dst_sb[:, 256:512], pps[1])
                return pps

            for it in range(T_NS - 1):
                mm256(M1, Pm, Pm)
                mm256(M2, M1, Pm)
                m3 = mm256(None, M2, sig)
                for b in range(2):
                    blk = ds(256 * b, 256)
                    nc.vector.tensor_scalar(
                        out=tmp256, in0=m3[b], scalar1=0.5, scalar2=None,
                        op0=ALU.mult)
                    nc.vector.tensor_scalar(
                        out=Pm[:, blk], in0=Pm[:, blk], scalar1=1.5, scalar2=None,
                        op0=ALU.mult)
                    nc.vector.tensor_sub(Pm[:, blk], Pm[:, blk], tmp256)

            # A_T = diag(rstd) * wm;  wm = P * sqrt(trace_inv)
            nc.vector.tensor_scalar_mul(acol_v, rstd_v, tis_t)
            for b in range(2):
                blk = ds(256 * b, 256)
                nc.vector.tensor_scalar_mul(A_T[:, blk], Pm[:, blk], acol_v[:, b:b + 1])
            # negb = -(A @ mu)
            for mb in range(2):
                nb = ps_tp.tile([128, 1], F32, tag="row")
                for kb in range(2):
                    nc.tensor.matmul(
                        nb, A_T[:, ds(256 * kb + 128 * mb, 128)].bitcast(F32), mu_v[:, kb:kb + 1],
                        start=(kb == 0), stop=(kb == 1), skip_group_check=True)
                nc.vector.tensor_scalar(
                    out=negb_v[:, mb:mb + 1], in0=nb, scalar1=-1.0, scalar2=None,
                    op0=ALU.mult)

        import os
        if os.environ.get("SKIP_STATS"):
            # bisection mode: A_T = I, negb = 0  ->  out == x
            nc.vector.tensor_scalar(
                out=A_T, in0=eps_eye, scalar1=1.0 / EPS, scalar2=None, op0=ALU.mult)
            nc.vector.memset(negb_v, 0.0)
        else:
            with tc.tile_pool(name="ps_ns", bufs=2, space="PSUM") as ps_ns:
                if reps > 1:
                    with tc.For_i(0, reps, 1):
                        stats_body(ps_ns)
                else:
                    stats_body(ps_ns)

        # =========================================================
        # PASS 2: out = A @ x + negb
        # =========================================================
        def pass2_body(stream, outp, ps_tp, _iv=None):
            atr = A_T

            def apply_chunks(xb0, xb1, src_off, n, dst_off, width):
                # process [dst_off, dst_off+width) of image n in P2C chunks
                ot0 = outp.tile([128, width], F32, tag="o0")
                ot1 = outp.tile([128, width], F32, tag="o1")
                for ci in range(width // P2C):
                    s = src_off + ci * P2C
                    o = ci * P2C
                    pa = ps_tp.tile([128, P2C], F32, tag="p2a")
                    pb = ps_tp.tile([128, P2C], F32, tag="p2b")
                    for mb, pp in ((0, pa), (1, pb)):
                        for kb, xb in ((0, xb0), (1, xb1)):
                            rhs = xb[:, ds(s, P2C)]
                            nc.tensor.matmul(
                                pp, atr[:, ds(256 * kb + 128 * mb, 128)], rhs,
                                start=(kb == 0), stop=(kb == 1),
                                skip_group_check=True)
                    nc.scalar.activation(
                        out=ot0[:, ds(o, P2C)], in_=pa, func=ACT.Identity,
                        bias=negb_v[:, 0:1], scale=1.0)
                    nc.vector.tensor_scalar(
                        out=ot1[:, ds(o, P2C)], in0=pb, scalar1=negb_v[:, 1:2],
                        scalar2=None, op0=ALU.add)
                for b, ot in ((0, ot0), (1, ot1)):
                    nc.sync.dma_start(
                        out=y[ds(n * C + 128 * b, 128), ds(dst_off, width)], in_=ot)

            for n in range(NPC):
                if n < RES_IMGS:
                    for half in range(2):
                        off = half * (HW // 2)
                        apply_chunks(xres[n][0], xres[n][1], off, n, off, HW // 2)
                else:
                    for w0 in range(0, HW, STREAM_W):
                        xs0 = stream.tile([128, STREAM_W], F32R, tag="xs0")
                        xs1 = stream.tile([128, STREAM_W], F32R, tag="xs1")
                        nc.sync.dma_start(
                            out=xs0, in_=x[ds(n * C, 128), ds(w0, STREAM_W)])
                        nc.sync.dma_start(
                            out=xs1, in_=x[ds(n * C + 128, 128), ds(w0, STREAM_W)])
                        apply_chunks(xs0, xs1, 0, n, w0, STREAM_W)

        with (
            tc.tile_pool(name="stream2", bufs=2) as stream2,
            tc.tile_pool(name="outp", bufs=2) as outp,
            tc.tile_pool(name="ps_p2", bufs=2, space="PSUM") as ps_p2,
        ):
            if reps > 1:
                with tc.For_i(0, reps, 1):
                    pass2_body(stream2, outp, ps_p2)
            else:
                pass2_body(stream2, outp, ps_p2)



def _build_split(phase):
    """phase='p1': pass1 + AllReduce -> g [128,520].
    phase='p2': x + A_T + negb -> y."""
    nc = bacc.Bacc("TRN2", target_bir_lowering=False, debug=False,
                   enable_asserts=False, num_devices=N_CORES)
    x = nc.dram_tensor("x", [NPC * C, HW], F32R, kind="ExternalInput").ap()
    if phase == "p1":
        g = nc.dram_tensor("g", [128, 520], F32, kind="ExternalOutput").ap()
    else:
        at_in = nc.dram_tensor("at", [128, 512], F32R, kind="ExternalInput").ap()
        nb_in = nc.dram_tensor("nb", [128, 2], F32, kind="ExternalInput").ap()
        y = nc.dram_tensor("y", [NPC * C, HW], F32, kind="ExternalOutput").ap()
    with tile.TileContext(nc) as tc:
        import contextlib
        ctx = contextlib.ExitStack()
        with ctx:
            consts = ctx.enter_context(tc.tile_pool(name="consts", bufs=1))
            resid = ctx.enter_context(tc.tile_pool(name="resid", bufs=1))
            stats = ctx.enter_context(tc.tile_pool(name="stats", bufs=1))
            dram = ctx.enter_context(tc.tile_pool(name="dram", bufs=1, space="DRAM"))
            ident32 = consts.tile([128, 128], F32)
            make_identity(nc, ident32)
            ones = consts.tile([128, 1], F32)
            nc.vector.memset(ones, 1.0)
            if phase == "p1":
                s_sb = stats.tile([128, 520], F32)
                ar_in = dram.tile([128, 520], F32)
                ar_out = dram.tile([128, 520], F32)
                with (
                    tc.tile_pool(name="stream1", bufs=4) as stream1,
                    tc.tile_pool(name="xtp", bufs=4) as xtp,
                    tc.tile_pool(name="ps_acc", bufs=1, space="PSUM") as ps_acc,
                    tc.tile_pool(name="ps_tp1", bufs=2, space="PSUM") as ps_tp1,
                ):
                    s_ps = [ps_acc.tile([128, 256], F32, tag=f"s{b}", name=f"s_ps{b}")
                            for b in range(2)]
                    rs_ps = [ps_acc.tile([128, 1], F32, tag=f"rs{b}", name=f"rs_ps{b}")
                             for b in range(2)]
                    first = [True]
                    n_chunks = NPC * (HW // P1C)
                    ci = [0]
                    for n in range(NPC):
                        for w0 in range(0, HW, STREAM_W):
                            xs0 = stream1.tile([128, STREAM_W], F32R, tag="xs0")
                            xs1 = stream1.tile([128, STREAM_W], F32R, tag="xs1")
                            nc.sync.dma_start(out=xs0, in_=x[ds(n * C, 128), ds(w0, STREAM_W)])
                            nc.sync.dma_start(out=xs1, in_=x[ds(n * C + 128, 128), ds(w0, STREAM_W)])
                            for s in range(0, STREAM_W, P1C):
                                tpA = ps_tp1.tile([128, 128], F32, tag="tpA")
                                tpB = ps_tp1.tile([128, 128], F32, tag="tpB")
                                nc.tensor.transpose(tpA[:P1C, :], xs0[:, ds(s, P1C)].bitcast(F32), ident32)
                                nc.tensor.transpose(tpB[:P1C, :], xs1[:, ds(s, P1C)].bitcast(F32), ident32)
                                xt = xtp.tile([128, 256], F32R, tag="xt")
                                nc.vector.tensor_copy(xt[:P1C, 0:128], tpA[:P1C, :])
                                nc.scalar.copy(xt[:P1C, 128:256], tpB[:P1C, :])
                                st = first[0]; first[0] = False
                                ci[0] += 1
                                sp = ci[0] == n_chunks
                                nc.tensor.matmul(s_ps[0], xt[:P1C, 0:128], xt[:P1C, 0:256],
                                                 start=st, stop=sp, skip_group_check=True)
                                nc.tensor.matmul(s_ps[1], xt[:P1C, 128:256], xt[:P1C, 0:256],
                                                 start=st, stop=sp, skip_group_check=True)
                                xtc = xt[:P1C, :].bitcast(F32)
                                nc.tensor.matmul(rs_ps[0], xtc[:, 0:128], ones[:P1C, :],
                                                 start=st, stop=sp, skip_group_check=True)
                                nc.tensor.matmul(rs_ps[1], xtc[:, 128:256], ones[:P1C, :],
                                                 start=st, stop=sp, skip_group_check=True)
                    nc.vector.tensor_copy(s_sb[:, 0:256], s_ps[0])
                    nc.scalar.copy(s_sb[:, 256:512], s_ps[1])
                    nc.vector.tensor_copy(s_sb[:, 512:513], rs_ps[0])
                    nc.vector.tensor_copy(s_sb[:, 513:514], rs_ps[1])
                    nc.vector.memset(s_sb[:, 514:520], 0.0)
                nc.sync.dma_start(out=ar_in, in_=s_sb)
                nc.gpsimd.collective_compute(
                    "AllReduce", ALU.add,
                    replica_groups=[list(range(N_CORES))],
                    ins=[ar_in.opt()], outs=[ar_out.opt()])
                nc.sync.dma_start(out=g, in_=ar_out)
            else:
                A_T = stats.tile([128, 512], F32R)
                negb_v = stats.tile([128, 2], F32)
                nc.sync.dma_start(out=A_T, in_=at_in)
                nc.sync.dma_start(out=negb_v, in_=nb_in)
                with (
                    tc.tile_pool(name="stream2", bufs=4) as stream2,
                    tc.tile_pool(name="outp", bufs=3) as outp,
                    tc.tile_pool(name="ps_p2", bufs=2, space="PSUM") as ps_p2,
                ):
                    for n in range(NPC):
                        for w0 in range(0, HW, STREAM_W):
                            xs0 = stream2.tile([128, STREAM_W], F32R, tag="xs0")
                            xs1 = stream2.tile([128, STREAM_W], F32R, tag="xs1")
                            nc.sync.dma_start(out=xs0, in_=x[ds(n * C, 128), ds(w0, STREAM_W)])
                            nc.sync.dma_start(out=xs1, in_=x[ds(n * C + 128, 128), ds(w0, STREAM_W)])
                            ot0 = outp.tile([128, STREAM_W], F32, tag="o0")
                            ot1 = outp.tile([128, STREAM_W], F32, tag="o1")
                            for ci2 in range(STREAM_W // P2C):
                                s = ci2 * P2C
                                pa = ps_p2.tile([128, P2C], F32, tag="p2a")
                                pb = ps_p2.tile([128, P2C], F32, tag="p2b")
                                for mb, pp in ((0, pa), (1, pb)):
                                    for kb, xb in ((0, xs0), (1, xs1)):
                                        nc.tensor.matmul(
                                            pp, A_T[:, ds(256 * kb + 128 * mb, 128)],
                                            xb[:, ds(s, P2C)], start=(kb == 0),
                                            stop=(kb == 1), skip_group_check=True)
                                nc.scalar.activation(
                                    out=ot0[:, ds(s, P2C)], in_=pa, func=ACT.Identity,
                                    bias=negb_v[:, 0:1], scale=1.0)
                                nc.vector.tensor_scalar(
                                    out=ot1[:, ds(s, P2C)], in0=pb, scalar1=negb_v[:, 1:2],
                                    scalar2=None, op0=ALU.add)
                            for b, ot in ((0, ot0), (1, ot1)):
                                nc.sync.dma_start(
                                    out=y[ds(n * C + 128 * b, 128), ds(w0, STREAM_W)], in_=ot)
    nc.compile()
    return nc


def _host_stats(g):
    S = np.empty((C, C), np.float64)
    S[0:128] = g[:, 0:256]; S[128:256] = g[:, 256:512]
    rs = np.empty(C, np.float64)
    rs[0:128] = g[:, 512]; rs[128:256] = g[:, 513]
    m = M_TOT
    mu = rs / m
    v = np.diag(S) - m * mu * mu
    std = np.sqrt(v / (m - 1)) + EPS_BN
    sigma = (S - m * np.outer(mu, mu)) / (m * np.outer(std, std)) + EPS * np.eye(C)
    ti = 1.0 / np.trace(sigma)
    sN = sigma * ti
    P = np.eye(C)
    for _ in range(T_NS):
        P = 1.5 * P - 0.5 * (P @ P @ P) @ sN
    wm = P * np.sqrt(ti)
    A_T = (wm / std[:, None])
    negb = -(A_T.T @ mu)
    at_sb = np.empty((128, 512), np.float32)
    at_sb[:, 0:256] = A_T[0:128]; at_sb[:, 256:512] = A_T[128:256]
    nb_sb = np.stack([negb[0:128], negb[128:256]], axis=1).astype(np.float32)
    return at_sb, nb_sb


def run_split(X):
    nc1 = _get_split("p1")
    in_maps = []
    shards = []
    for r in range(N_CORES):
        sh = np.ascontiguousarray(X[r * NPC:(r + 1) * NPC]).reshape(NPC * C, HW)
        shards.append(sh)
        in_maps.append({"x": sh})
    res1 = run_bass_kernel_spmd(nc1, in_maps, core_ids=list(range(N_CORES)), trace=False)
    g = res1.results[0]["g"].astype(np.float64)
    at_sb, nb_sb = _host_stats(g)
    nc2 = _get_split("p2")
    in_maps2 = [{"x": shards[r], "at": at_sb, "nb": nb_sb} for r in range(N_CORES)]
    res2 = run_bass_kernel_spmd(nc2, in_maps2, core_ids=list(range(N_CORES)), trace=False)
    out = np.empty((N, C, H, W), dtype=np.float32)
    for r in range(N_CORES):
        out[r * NPC:(r + 1) * NPC] = res2.results[r]["y"].reshape(NPC, C, H, W)
    return out


_SPLIT_CACHE = {}


def _get_split(phase):
    if phase not in _SPLIT_CACHE:
        _SPLIT_CACHE[phase] = _build_split(phase)
    return _SPLIT_CACHE[phase]


_CACHE = {}


def get_nc(reps: int = 1):
    if reps not in _CACHE:
        _CACHE[reps] = _build(reps)
    return _CACHE[reps]


def run(X: np.ndarray, reps: int = 1):
    nc = get_nc(reps)
    in_maps = []
    for r in range(N_CORES):
        shard = np.ascontiguousarray(X[r * NPC:(r + 1) * NPC]).reshape(NPC * C, HW)
        in_maps.append({"x": shard})
    res = run_bass_kernel_spmd(
        nc, in_maps, core_ids=list(range(N_CORES)), trace=False)
    out = np.empty((N, C, H, W), dtype=np.float32)
    for r in range(N_CORES):
        out[r * NPC:(r + 1) * NPC] = res.results[r]["y"].reshape(NPC, C, H, W)
    return out


def kernel(X: np.ndarray) -> np.ndarray:
    import os
    if os.environ.get("FUSED_KERNEL"):
        return run(np.asarray(X, dtype=np.float32), reps=1)
    return run_split(np.asarray(X, dtype=np.float32))

